# revision 6
# baseline (speedup 1.0000x reference)
"""Causal multi-head attention forward on 8 Trainium2 NeuronCores.

Sharding: head-parallel (tensor parallel). 16 heads / 8 cores = 2 heads per
core. Each core:
  - computes q/k/v projections for its 2 heads (weight-sliced),
  - runs causal attention for its 4 (batch, head) pairs,
  - computes the partial output projection over its 128 head-dims.
The host sums the 8 partial outputs (the unshard for a contraction-sharded
matmul) and adds the output bias.

Device design (bf16 matmuls, fp32 PSUM accumulate):
  xT      [1024, 4096]       x transposed (contraction dim on partitions)
  qT, kT  [128, 4096]        head-dims on partitions (q pre-scaled by 1/8)
  v       [128s, 16, 2, 128]  per batch: per sk-tile, per head, an augmented
                              [sk, 128] operand: h0 = [v(64) | ones | zeros],
                              h1 = [ones | zeros | v(64)]. The ones column
                              makes the attention@v matmul emit the softmax
                              denominator as an extra output row (row 64 for
                              h0, row 0 for h1), and places h1's context on
                              partitions 64..127 directly.
  scores  [sk=128, 2, sq<=512] joint PSUM tile (both heads side by side) so
                              one exp instruction covers both heads; the two
                              score matmuls sit on PE row-groups 0-63/64-127
                              and overlap in the array.
Causal handling: fully-masked column ranges are never computed (matmuls and
exp stream columns [r:512] only); the 128-wide diagonal band is handled by
multiplying exp(scores) with a 0/1 mask on the otherwise idle GpSimd engine.
Softmax normalization: reciprocal of the denominator rows on DVE, broadcast
across partitions with a DRAM-bounce DMA, one multiply per head half.
"""

import os

import numpy as np
import ml_dtypes

D_MODEL = 1024
N_HEAD = 16
D_K = 64
B = 2
S = 2048
SF = B * S  # 4096 flattened tokens
N_CORES = 8
HPC = 2  # heads per core
C_LOC = HPC * D_K  # 128 head-dims per core

BF16 = ml_dtypes.bfloat16

_cache = {}


def _build_program():
    """Build + compile the single-core SPMD Bass program (identical on all
    cores; per-core behavior comes entirely from the input tensors)."""
    import concourse.bass as bass  # noqa: F401
    import concourse.tile as tile
    from concourse import bacc, mybir

    fp32 = mybir.dt.float32
    bf16 = mybir.dt.bfloat16
    EXP = mybir.ActivationFunctionType.Exp

    nc = bacc.Bacc(
        "TRN2",
        target_bir_lowering=False,
        debug=False,
        enable_asserts=False,
        num_devices=N_CORES,
    )

    xt_d = nc.dram_tensor("xt", [D_MODEL, SF], bf16, kind="ExternalInput").ap()
    wq_d = nc.dram_tensor("wq", [128, 8, 128], bf16, kind="ExternalInput").ap()
    wk_d = nc.dram_tensor("wk", [128, 8, 128], bf16, kind="ExternalInput").ap()
    wv_d = nc.dram_tensor("wv", [128, 8, 128], bf16, kind="ExternalInput").ap()
    wo_d = nc.dram_tensor("wo", [128, D_MODEL], bf16, kind="ExternalInput").ap()
    bq_d = nc.dram_tensor("bq", [128, 1], fp32, kind="ExternalInput").ap()
    bk_d = nc.dram_tensor("bk", [128, 1], fp32, kind="ExternalInput").ap()
    mask_d = nc.dram_tensor("mask", [128, 128], bf16, kind="ExternalInput").ap()
    out_d = nc.dram_tensor("out", [SF, D_MODEL], fp32, kind="ExternalOutput").ap()

    with tile.TileContext(nc) as tc:
        with (
            tc.tile_pool(name="persist", bufs=1) as persist,
            tc.tile_pool(name="work", bufs=2) as work,
            tc.tile_pool(name="psum", bufs=1, space="PSUM") as psum,
            tc.tile_pool(name="dscratch", bufs=2, space="DRAM") as dpool,
        ):
            # ---- resident weights/constants -------------------------------
            wq_sb = persist.tile([128, 8, 128], bf16, tag="wq")
            wk_sb = persist.tile([128, 8, 128], bf16, tag="wk")
            wv_sb = persist.tile([128, 8, 128], bf16, tag="wv")
            wo_sb = persist.tile([128, D_MODEL], bf16, tag="wo")
            bq_sb = persist.tile([128, 1], fp32, tag="bq")
            bk_sb = persist.tile([128, 1], fp32, tag="bk")
            mask_sb = persist.tile([128, 128], bf16, tag="mask")
            nc.sync.dma_start(out=wq_sb[:], in_=wq_d[:])
            nc.sync.dma_start(out=wk_sb[:], in_=wk_d[:])
            nc.sync.dma_start(out=wv_sb[:], in_=wv_d[:])
            nc.sync.dma_start(out=wo_sb[:], in_=wo_d[:])
            nc.sync.dma_start(out=bq_sb[:], in_=bq_d[:])
            nc.sync.dma_start(out=bk_sb[:], in_=bk_d[:])
            nc.sync.dma_start(out=mask_sb[:], in_=mask_d[:])

            # ---- x transposed, resident ----------------------------------
            xt_sb = []
            for kk in range(8):
                t = persist.tile([128, SF], bf16, tag=f"xt{kk}", name=f"xt{kk}")
                nc.sync.dma_start(out=t[:], in_=xt_d[kk * 128 : (kk + 1) * 128, :])
                xt_sb.append(t)

            # ---- projection outputs, resident ----------------------------
            qt_sb = persist.tile([128, SF], bf16, tag="qt")
            kt_sb = persist.tile([128, SF], bf16, tag="kt")
            v_sb = []
            for b in range(B):
                t = persist.tile([128, 16, 2, 128], bf16, tag=f"v{b}", name=f"v{b}")
                # h0 block: [v(0:64) | ones col 64 | zeros 65:128]
                nc.vector.memset(t[:, :, 0, 64:65], 1.0)
                nc.vector.memset(t[:, :, 0, 65:128], 0.0)
                # h1 block: [ones col 0 | zeros 1:64 | v(64:128)]
                nc.vector.memset(t[:, :, 1, 0:1], 1.0)
                nc.vector.memset(t[:, :, 1, 1:64], 0.0)
                v_sb.append(t)

            # ---- fused schedule --------------------------------------------
            # Pair t = (projection chunk t) + (o-proj of attention block t-1)
            # + (attention block t). Interleaving keeps the PE stream dense
            # (PE-heavy projection fills the exp-wait gaps of the ACT-heavy
            # attention), which also keeps the HAM clock-gate at 2.4 GHz.
            def proj_chunk(n):
                cs = slice(n * 512, (n + 1) * 512)
                qk = psum.tile([128, 2, 512], fp32, tag="st", bufs=2, name="qk")
                vj = psum.tile([128, 4, 128], fp32, tag="av", bufs=2, name="vj")
                for kk in range(8):
                    st_flag = kk == 0
                    sp_flag = kk == 7
                    nc.tensor.matmul(
                        qk[:, 0, :],
                        lhsT=wq_sb[:, kk, :],
                        rhs=xt_sb[kk][:, cs],
                        start=st_flag,
                        stop=sp_flag,
                    )
                    for ss in (0, 1):
                        t128 = n * 4 + ss
                        # all four v slices live in one PSUM bank: one
                        # accumulation group spans them (start clears the
                        # bank; has_written handles first-touch overwrite)
                        nc.tensor.matmul(
                            vj[:, ss, :],
                            lhsT=xt_sb[kk][:, t128 * 128 : (t128 + 1) * 128],
                            rhs=wv_sb[:, kk, :],
                            start=(kk == 0 and ss == 0),
                            stop=(kk == 7 and ss == 3),
                        )
                    nc.tensor.matmul(
                        qk[:, 1, :],
                        lhsT=wk_sb[:, kk, :],
                        rhs=xt_sb[kk][:, cs],
                        start=st_flag,
                        stop=sp_flag,
                    )
                    for ss in (2, 3):
                        t128 = n * 4 + ss
                        nc.tensor.matmul(
                            vj[:, ss, :],
                            lhsT=xt_sb[kk][:, t128 * 128 : (t128 + 1) * 128],
                            rhs=wv_sb[:, kk, :],
                            start=False,
                            stop=(kk == 7 and ss == 3),
                        )
                nc.vector.tensor_scalar_add(
                    out=qt_sb[:, cs], in0=qk[:, 0, :], scalar1=bq_sb
                )
                nc.vector.tensor_scalar_add(
                    out=kt_sb[:, cs], in0=qk[:, 1, :], scalar1=bk_sb
                )
                for ss in range(4):
                    t128 = n * 4 + ss
                    b, i = divmod(t128, 16)
                    nc.vector.tensor_copy(
                        out=v_sb[b][:, i, 0, 0:64], in_=vj[:, ss, 0:64]
                    )
                    nc.vector.tensor_copy(
                        out=v_sb[b][:, i, 1, 64:128], in_=vj[:, ss, 64:128]
                    )

            def attn_block(b, J):
                """Scores+exp+attention@v for one (batch, 512-query) block.
                Returns the normalized-context tile for the deferred o-proj.
                The attention@v matmuls trail the score matmuls by AV_LAG
                iterations so the PE never waits on the ScalarE exp."""
                AV_LAG = 3
                cb = b * S
                av = psum.tile([128, 2, 512], fp32, tag="av", bufs=2, name="av")
                ntiles = 4 * (J + 1)
                ptiles = {}

                def do_av(i):
                    r = max(0, i * 128 - J * 512)
                    p = ptiles.pop(i)
                    for h in range(2):
                        nc.tensor.matmul(
                            av[:, h, r:512],
                            lhsT=v_sb[b][:, i, h, :],
                            rhs=p[:, h, r:512],
                            start=(i == 0),
                            stop=(i == ntiles - 1),
                        )

                for i in range(ntiles):
                    r = max(0, i * 128 - J * 512)  # 0 except diagonal tiles
                    diag = i >= 4 * J
                    ks = slice(cb + i * 128, cb + (i + 1) * 128)
                    qs = slice(cb + J * 512 + r, cb + (J + 1) * 512)
                    st = psum.tile(
                        [128, 2, 512], fp32, tag="st", bufs=2, name="st"
                    )
                    for h in range(2):
                        hp = slice(h * 64, (h + 1) * 64)
                        nc.tensor.matmul(
                            st[:, h, r:512],
                            lhsT=kt_sb[hp, ks],
                            rhs=qt_sb[hp, qs],
                            start=True,
                            stop=True,
                        )
                    p = work.tile([128, 2, 512], bf16, tag="p", bufs=6, name="p")
                    nc.scalar.activation(
                        out=p[:, :, r:512], in_=st[:, :, r:512], func=EXP
                    )
                    if diag:
                        # zero the not-allowed part of the 128-wide band
                        for h in range(2):
                            nc.gpsimd.tensor_mul(
                                out=p[:, h, r : r + 128],
                                in0=p[:, h, r : r + 128],
                                in1=mask_sb[:],
                            )
                    ptiles[i] = p
                    if i >= AV_LAG:
                        do_av(i - AV_LAG)
                for i in range(max(0, ntiles - AV_LAG), ntiles):
                    do_av(i)

                # Drain the attention output out of PSUM immediately so the
                # av slot frees fast (the normalization chain below has DMA
                # latency in it and must not gate PSUM reuse).
                avu = work.tile([128, 2, 512], fp32, tag="avu", bufs=2, name="avu")
                nc.vector.tensor_copy(out=avu[0:65, 0, :], in_=av[0:65, 0, :])
                nc.vector.tensor_copy(out=avu[:, 1, :], in_=av[:, 1, :])

                # softmax denominators: h0 on partition 64, h1 on partition 0.
                # DVE reciprocal costs ~6 cycles/elem/lane, so reshape the
                # [1,512] rows to [128,4] via DMA and do one tiny reciprocal.
                rdd = dpool.tile([2, 512], fp32, tag="rdd", bufs=2, name="rdd")
                nc.sync.dma_start(out=rdd[0:1, :], in_=avu[64:65, 0, :])
                nc.sync.dma_start(out=rdd[1:2, :], in_=avu[0:1, 1, :])
                dd = work.tile([128, 8], fp32, tag="dd", bufs=2, name="dd")
                nc.sync.dma_start(
                    out=dd.rearrange("p (h m) -> p h m", h=2),
                    in_=rdd[0:2, :].rearrange("h (m p) -> p h m", p=128),
                )
                ddr = work.tile([128, 8], fp32, tag="ddr", bufs=2, name="ddr")
                nc.vector.reciprocal(out=ddr[:], in_=dd[:])
                rd = dpool.tile([2, 512], fp32, tag="rd", bufs=2, name="rd")
                nc.sync.dma_start(
                    out=rd[0:2, :].rearrange("h (m p) -> p h m", p=128),
                    in_=ddr.rearrange("p (h m) -> p h m", h=2),
                )
                rb = work.tile([128, 512], fp32, tag="rb", bufs=2, name="rb")
                nc.sync.dma_start(
                    out=rb[0:64, :], in_=rd[0:1, :].to_broadcast([64, 512])
                )
                nc.sync.dma_start(
                    out=rb[64:128, :], in_=rd[1:2, :].to_broadcast([64, 512])
                )
                # normalized context, bf16, head-dims on partitions; all
                # operands are in SBUF so the idle GpSimd engine does this
                ctxt = work.tile([128, 512], bf16, tag="ctx", bufs=2, name="ctxt")
                nc.gpsimd.tensor_mul(
                    out=ctxt[0:64, :], in0=avu[0:64, 0, :], in1=rb[0:64, :]
                )
                nc.gpsimd.tensor_mul(
                    out=ctxt[64:128, :], in0=avu[64:128, 1, :], in1=rb[64:128, :]
                )
                return ctxt

            def oproj_block(b, J, ctxt):
                # partial output projection for these 512 tokens
                for m in range(4):
                    op = psum.tile([128, 2, 512], fp32, tag="av", bufs=2, name="op")
                    for nn in range(2):
                        nc.tensor.matmul(
                            op[:, nn, :],
                            lhsT=ctxt[:, m * 128 : (m + 1) * 128],
                            rhs=wo_sb[:, nn * 512 : (nn + 1) * 512],
                            start=True,
                            stop=True,
                        )
                    ob = work.tile([128, D_MODEL], fp32, tag="ob", bufs=3, name="ob")
                    nc.vector.tensor_copy(out=ob[:], in_=op[:])
                    row0 = b * S + J * 512 + m * 128
                    nc.sync.dma_start(out=out_d[row0 : row0 + 128, :], in_=ob[:])

            pending = None  # (b, J, ctxt) awaiting o-proj
            for t in range(8):
                proj_chunk(t)
                if pending is not None:
                    oproj_block(*pending)
                b, J = divmod(t, 4)
                pending = (b, J, attn_block(b, J))
            oproj_block(*pending)

    nc.compile()
    return nc


def get_program():
    if "nc" not in _cache:
        _cache["nc"] = _build_program()
    return _cache["nc"]


def shard_inputs(x, Wq, bq, Wk, bk, Wv, bv, Wo, bo):
    """Host-side sharding/layout prep. Returns (in_maps, bo_eff)."""
    x = np.asarray(x, dtype=np.float32)
    Wq = np.asarray(Wq, dtype=np.float32)
    Wk = np.asarray(Wk, dtype=np.float32)
    Wv = np.asarray(Wv, dtype=np.float32)
    Wo = np.asarray(Wo, dtype=np.float32)
    bq = np.asarray(bq, dtype=np.float32)
    bk = np.asarray(bk, dtype=np.float32)
    bv = np.asarray(bv, dtype=np.float32)
    bo = np.asarray(bo, dtype=np.float32)

    xt = np.ascontiguousarray(x.reshape(SF, D_MODEL).T).astype(BF16)
    # 0/1 mask for the diagonal band: allowed iff sk(partition) <= sq(col)
    mask = (
        (np.arange(128)[:, None] <= np.arange(128)[None, :])
        .astype(np.float32)
        .astype(BF16)
    )
    # v-bias passes through attention unchanged (attn rows sum to 1), so it
    # folds into the output bias: bo_eff = bo + Wo @ bv.
    bo_eff = bo + Wo @ bv

    def pack_lhsT(w):  # [1024, 128] k-major -> [128, 8, 128] (p, kk, m)
        return np.ascontiguousarray(
            w.reshape(8, 128, 128).transpose(1, 0, 2)
        ).astype(BF16)

    in_maps = []
    for c in range(N_CORES):
        rows = slice(c * C_LOC, (c + 1) * C_LOC)
        in_maps.append(
            {
                "xt": xt,
                "wq": pack_lhsT((Wq[rows, :] / 8.0).T),
                "wk": pack_lhsT(Wk[rows, :].T),
                "wv": pack_lhsT(Wv[rows, :].T),
                "wo": np.ascontiguousarray(Wo[:, rows].T).astype(BF16),
                "bq": (bq[rows] / 8.0).reshape(128, 1).astype(np.float32),
                "bk": bk[rows].reshape(128, 1).astype(np.float32),
                "mask": mask,
            }
        )
    return in_maps, bo_eff


LAST_RESULTS = None  # BassKernelResults of the most recent run
LAST_RUN_WALL_S = None  # wall seconds of the most recent device dispatch


def kernel(x, Wq, bq, Wk, bk, Wv, bv, Wo, bo):
    global LAST_RESULTS, LAST_RUN_WALL_S
    import time

    from concourse.bass_utils import run_bass_kernel_spmd

    nc = get_program()
    in_maps, bo_eff = shard_inputs(x, Wq, bq, Wk, bk, Wv, bv, Wo, bo)
    trace = bool(os.environ.get("ATTN_KERNEL_TRACE"))
    t0 = time.time()
    res = run_bass_kernel_spmd(
        nc,
        in_maps,
        list(range(N_CORES)),
        trace=trace,
        trace_cores=list(range(N_CORES)) if trace else None,
    )
    LAST_RUN_WALL_S = time.time() - t0
    LAST_RESULTS = res
    acc = np.zeros((SF, D_MODEL), dtype=np.float32)
    for r in res.results:
        acc += np.asarray(r["out"], dtype=np.float32)
    acc += bo_eff[None, :]
    return acc.reshape(B, S, D_MODEL).astype(np.float32)


# revision 7
# speedup vs baseline: 1.0516x; 1.0516x over previous
"""Causal multi-head attention forward on 8 Trainium2 NeuronCores.

Sharding: head-parallel (tensor parallel). 16 heads / 8 cores = 2 heads per
core. Each core:
  - computes q/k/v projections for its 2 heads (weight-sliced),
  - runs causal attention for its 4 (batch, head) pairs,
  - computes the partial output projection over its 128 head-dims.
The host sums the 8 partial outputs (the unshard for a contraction-sharded
matmul) and adds the output bias.

Device design (bf16 matmuls, fp32 PSUM accumulate):
  xT      [1024, 4096]       x transposed (contraction dim on partitions)
  qT, kT  [128, 4096]        head-dims on partitions (q pre-scaled by 1/8)
  v       [128s, 16, 2, 128]  per batch: per sk-tile, per head, an augmented
                              [sk, 128] operand: h0 = [v(64) | ones | zeros],
                              h1 = [ones | zeros | v(64)]. The ones column
                              makes the attention@v matmul emit the softmax
                              denominator as an extra output row (row 64 for
                              h0, row 0 for h1), and places h1's context on
                              partitions 64..127 directly.
  scores  [sk=128, 2, sq<=512] joint PSUM tile (both heads side by side) so
                              one exp instruction covers both heads; the two
                              score matmuls sit on PE row-groups 0-63/64-127
                              and overlap in the array.
Causal handling: fully-masked column ranges are never computed (matmuls and
exp stream columns [r:512] only); the 128-wide diagonal band is handled by
multiplying exp(scores) with a 0/1 mask on the otherwise idle GpSimd engine.
Softmax normalization: reciprocal of the denominator rows on DVE, broadcast
across partitions with a DRAM-bounce DMA, one multiply per head half.
"""

import os

import numpy as np
import ml_dtypes

D_MODEL = 1024
N_HEAD = 16
D_K = 64
B = 2
S = 2048
SF = B * S  # 4096 flattened tokens
N_CORES = 8
HPC = 2  # heads per core
C_LOC = HPC * D_K  # 128 head-dims per core

BF16 = ml_dtypes.bfloat16

_cache = {}


def _build_program():
    """Build + compile the single-core SPMD Bass program (identical on all
    cores; per-core behavior comes entirely from the input tensors)."""
    import concourse.bass as bass  # noqa: F401
    import concourse.tile as tile
    from concourse import bacc, mybir

    fp32 = mybir.dt.float32
    bf16 = mybir.dt.bfloat16
    EXP = mybir.ActivationFunctionType.Exp

    nc = bacc.Bacc(
        "TRN2",
        target_bir_lowering=False,
        debug=False,
        enable_asserts=False,
        num_devices=N_CORES,
    )

    xt_d = nc.dram_tensor("xt", [D_MODEL, SF], bf16, kind="ExternalInput").ap()
    wq_d = nc.dram_tensor("wq", [128, 8, 128], bf16, kind="ExternalInput").ap()
    wk_d = nc.dram_tensor("wk", [128, 8, 128], bf16, kind="ExternalInput").ap()
    wv_d = nc.dram_tensor("wv", [128, 8, 128], bf16, kind="ExternalInput").ap()
    wo_d = nc.dram_tensor("wo", [128, D_MODEL], bf16, kind="ExternalInput").ap()
    bq_d = nc.dram_tensor("bq", [128, 1], fp32, kind="ExternalInput").ap()
    bk_d = nc.dram_tensor("bk", [128, 1], fp32, kind="ExternalInput").ap()
    mask_d = nc.dram_tensor("mask", [128, 128], bf16, kind="ExternalInput").ap()
    out_d = nc.dram_tensor("out", [SF, D_MODEL], fp32, kind="ExternalOutput").ap()

    with tile.TileContext(nc) as tc:
        with (
            tc.tile_pool(name="persist", bufs=1) as persist,
            tc.tile_pool(name="work", bufs=2) as work,
            tc.tile_pool(name="psum", bufs=1, space="PSUM") as psum,
            tc.tile_pool(name="dscratch", bufs=2, space="DRAM") as dpool,
        ):
            # ---- resident weights/constants -------------------------------
            wq_sb = persist.tile([128, 8, 128], bf16, tag="wq")
            wk_sb = persist.tile([128, 8, 128], bf16, tag="wk")
            wv_sb = persist.tile([128, 8, 128], bf16, tag="wv")
            wo_sb = persist.tile([128, D_MODEL], bf16, tag="wo")
            bq_sb = persist.tile([128, 1], fp32, tag="bq")
            bk_sb = persist.tile([128, 1], fp32, tag="bk")
            mask_sb = persist.tile([128, 128], bf16, tag="mask")
            nc.sync.dma_start(out=wq_sb[:], in_=wq_d[:])
            nc.sync.dma_start(out=wk_sb[:], in_=wk_d[:])
            nc.sync.dma_start(out=wv_sb[:], in_=wv_d[:])
            nc.sync.dma_start(out=wo_sb[:], in_=wo_d[:])
            nc.sync.dma_start(out=bq_sb[:], in_=bq_d[:])
            nc.sync.dma_start(out=bk_sb[:], in_=bk_d[:])
            nc.sync.dma_start(out=mask_sb[:], in_=mask_d[:])

            # ---- x transposed, resident ----------------------------------
            xt_sb = []
            for kk in range(8):
                t = persist.tile([128, SF], bf16, tag=f"xt{kk}", name=f"xt{kk}")
                nc.sync.dma_start(out=t[:], in_=xt_d[kk * 128 : (kk + 1) * 128, :])
                xt_sb.append(t)

            # ---- projection outputs, resident ----------------------------
            qt_sb = persist.tile([128, SF], bf16, tag="qt")
            kt_sb = persist.tile([128, SF], bf16, tag="kt")
            v_sb = []
            for b in range(B):
                t = persist.tile([128, 16, 2, 128], bf16, tag=f"v{b}", name=f"v{b}")
                # h0 block: [v(0:64) | ones col 64 | zeros 65:128]
                nc.vector.memset(t[:, :, 0, 64:65], 1.0)
                nc.vector.memset(t[:, :, 0, 65:128], 0.0)
                # h1 block: [ones col 0 | zeros 1:64 | v(64:128)]
                nc.vector.memset(t[:, :, 1, 0:1], 1.0)
                nc.vector.memset(t[:, :, 1, 1:64], 0.0)
                v_sb.append(t)

            # ---- fused schedule --------------------------------------------
            # Pair t = (projection chunk t) + (o-proj of attention block t-1)
            # + (attention block t). Interleaving keeps the PE stream dense
            # (PE-heavy projection fills the exp-wait gaps of the ACT-heavy
            # attention), which also keeps the HAM clock-gate at 2.4 GHz.
            def proj_chunk(n):
                cs = slice(n * 512, (n + 1) * 512)
                qk = psum.tile([128, 2, 512], fp32, tag="st", bufs=2, name="qk")
                vj = psum.tile([128, 4, 128], fp32, tag="av", bufs=2, name="vj")
                for kk in range(8):
                    st_flag = kk == 0
                    sp_flag = kk == 7
                    nc.tensor.matmul(
                        qk[:, 0, :],
                        lhsT=wq_sb[:, kk, :],
                        rhs=xt_sb[kk][:, cs],
                        start=st_flag,
                        stop=sp_flag,
                    )
                    for ss in (0, 1):
                        t128 = n * 4 + ss
                        # all four v slices live in one PSUM bank: one
                        # accumulation group spans them (start clears the
                        # bank; has_written handles first-touch overwrite)
                        nc.tensor.matmul(
                            vj[:, ss, :],
                            lhsT=xt_sb[kk][:, t128 * 128 : (t128 + 1) * 128],
                            rhs=wv_sb[:, kk, :],
                            start=(kk == 0 and ss == 0),
                            stop=(kk == 7 and ss == 3),
                        )
                    nc.tensor.matmul(
                        qk[:, 1, :],
                        lhsT=wk_sb[:, kk, :],
                        rhs=xt_sb[kk][:, cs],
                        start=st_flag,
                        stop=sp_flag,
                    )
                    for ss in (2, 3):
                        t128 = n * 4 + ss
                        nc.tensor.matmul(
                            vj[:, ss, :],
                            lhsT=xt_sb[kk][:, t128 * 128 : (t128 + 1) * 128],
                            rhs=wv_sb[:, kk, :],
                            start=False,
                            stop=(kk == 7 and ss == 3),
                        )
                nc.vector.tensor_scalar_add(
                    out=qt_sb[:, cs], in0=qk[:, 0, :], scalar1=bq_sb
                )
                nc.vector.tensor_scalar_add(
                    out=kt_sb[:, cs], in0=qk[:, 1, :], scalar1=bk_sb
                )
                for ss in range(4):
                    t128 = n * 4 + ss
                    b, i = divmod(t128, 16)
                    nc.vector.tensor_copy(
                        out=v_sb[b][:, i, 0, 0:64], in_=vj[:, ss, 0:64]
                    )
                    nc.vector.tensor_copy(
                        out=v_sb[b][:, i, 1, 64:128], in_=vj[:, ss, 64:128]
                    )

            def attn_block(b, J):
                """Scores+exp+attention@v for one (batch, 512-query) block.
                Returns the normalized-context tile for the deferred o-proj.
                The attention@v matmuls trail the score matmuls by AV_LAG
                iterations so the PE never waits on the ScalarE exp."""
                AV_LAG = 3
                cb = b * S
                av = psum.tile([128, 2, 512], fp32, tag="av", bufs=2, name="av")
                ntiles = 4 * (J + 1)
                ptiles = {}

                def do_av(i):
                    r = max(0, i * 128 - J * 512)
                    p = ptiles.pop(i)
                    for h in range(2):
                        nc.tensor.matmul(
                            av[:, h, r:512],
                            lhsT=v_sb[b][:, i, h, :],
                            rhs=p[:, h, r:512],
                            start=(i == 0),
                            stop=(i == ntiles - 1),
                        )

                for i in range(ntiles):
                    r = max(0, i * 128 - J * 512)  # 0 except diagonal tiles
                    diag = i >= 4 * J
                    ks = slice(cb + i * 128, cb + (i + 1) * 128)
                    qs = slice(cb + J * 512 + r, cb + (J + 1) * 512)
                    st = psum.tile(
                        [128, 2, 512], fp32, tag="st", bufs=2, name="st"
                    )
                    for h in range(2):
                        hp = slice(h * 64, (h + 1) * 64)
                        nc.tensor.matmul(
                            st[:, h, r:512],
                            lhsT=kt_sb[hp, ks],
                            rhs=qt_sb[hp, qs],
                            start=True,
                            stop=True,
                        )
                    p = work.tile([128, 2, 512], bf16, tag="p", bufs=6, name="p")
                    nc.scalar.activation(
                        out=p[:, :, r:512], in_=st[:, :, r:512], func=EXP
                    )
                    if diag:
                        # zero the not-allowed part of the 128-wide band
                        for h in range(2):
                            nc.gpsimd.tensor_mul(
                                out=p[:, h, r : r + 128],
                                in0=p[:, h, r : r + 128],
                                in1=mask_sb[:],
                            )
                    ptiles[i] = p
                    if i >= AV_LAG:
                        do_av(i - AV_LAG)
                for i in range(max(0, ntiles - AV_LAG), ntiles):
                    do_av(i)

                # Drain the attention output out of PSUM immediately so the
                # av slot frees fast (the normalization chain has DMA latency
                # in it and must not gate PSUM reuse).
                avu = work.tile([128, 2, 512], fp32, tag="avu", bufs=3, name="avu")
                nc.vector.tensor_copy(out=avu[0:65, 0, :], in_=av[0:65, 0, :])
                nc.vector.tensor_copy(out=avu[:, 1, :], in_=av[:, 1, :])

                # softmax denominators: h0 on partition 64, h1 on partition 0.
                # DVE reciprocal costs ~6 cycles/elem/lane, so reshape the
                # [1,512] rows to [128,4] via a DRAM bounce and do one tiny
                # reciprocal. Every consumer of this chain is traced 1-2
                # pairs later so no engine FIFO ever blocks on its latency.
                rdd = dpool.tile([2, 512], fp32, tag="rdd", bufs=3, name="rdd")
                nc.sync.dma_start(out=rdd[0:1, :], in_=avu[64:65, 0, :])
                nc.sync.dma_start(out=rdd[1:2, :], in_=avu[0:1, 1, :])
                dd = work.tile([128, 8], fp32, tag="dd", bufs=3, name="dd")
                nc.sync.dma_start(
                    out=dd.rearrange("p (h m) -> p h m", h=2),
                    in_=rdd[0:2, :].rearrange("h (m p) -> p h m", p=128),
                )
                return avu, dd

            def norm_block(state):
                # one pair after attn_block: reciprocal + broadcast set-up
                avu, dd = state
                ddr = work.tile([128, 8], fp32, tag="ddr", bufs=3, name="ddr")
                nc.vector.reciprocal(out=ddr[:], in_=dd[:])
                rd = dpool.tile([2, 512], fp32, tag="rd", bufs=3, name="rd")
                nc.sync.dma_start(
                    out=rd[0:2, :].rearrange("h (m p) -> p h m", p=128),
                    in_=ddr.rearrange("p (h m) -> p h m", h=2),
                )
                rb = work.tile([128, 512], fp32, tag="rb", bufs=3, name="rb")
                nc.sync.dma_start(
                    out=rb[0:64, :], in_=rd[0:1, :].to_broadcast([64, 512])
                )
                nc.sync.dma_start(
                    out=rb[64:128, :], in_=rd[1:2, :].to_broadcast([64, 512])
                )
                return avu, rb

            def oproj_block(b, J, state):
                # two pairs after attn_block: normalize (on GpSimd — all
                # operands in SBUF) and do the partial output projection.
                avu, rb = state
                ctxt = work.tile([128, 512], bf16, tag="ctx", bufs=2, name="ctxt")
                nc.gpsimd.tensor_mul(
                    out=ctxt[0:64, :], in0=avu[0:64, 0, :], in1=rb[0:64, :]
                )
                nc.gpsimd.tensor_mul(
                    out=ctxt[64:128, :], in0=avu[64:128, 1, :], in1=rb[64:128, :]
                )
                for m in range(4):
                    op = psum.tile([128, 2, 512], fp32, tag="av", bufs=2, name="op")
                    for nn in range(2):
                        nc.tensor.matmul(
                            op[:, nn, :],
                            lhsT=ctxt[:, m * 128 : (m + 1) * 128],
                            rhs=wo_sb[:, nn * 512 : (nn + 1) * 512],
                            start=True,
                            stop=True,
                        )
                    ob = work.tile([128, D_MODEL], fp32, tag="ob", bufs=3, name="ob")
                    nc.vector.tensor_copy(out=ob[:], in_=op[:])
                    row0 = b * S + J * 512 + m * 128
                    nc.sync.dma_start(out=out_d[row0 : row0 + 128, :], in_=ob[:])

            # 3-stage pipeline over the 8 pairs: attn(t) | norm(t-1) |
            # normalize+oproj(t-2), with projection chunk t leading pair t.
            states = {}
            for t in range(10):
                if t < 8:
                    proj_chunk(t)
                if 1 <= t <= 8:
                    states[t - 1] = norm_block(states[t - 1])
                if t >= 2:
                    b, J = divmod(t - 2, 4)
                    oproj_block(b, J, states.pop(t - 2))
                if t < 8:
                    b, J = divmod(t, 4)
                    states[t] = attn_block(b, J)

    nc.compile()
    return nc


def get_program():
    if "nc" not in _cache:
        _cache["nc"] = _build_program()
    return _cache["nc"]


def shard_inputs(x, Wq, bq, Wk, bk, Wv, bv, Wo, bo):
    """Host-side sharding/layout prep. Returns (in_maps, bo_eff)."""
    x = np.asarray(x, dtype=np.float32)
    Wq = np.asarray(Wq, dtype=np.float32)
    Wk = np.asarray(Wk, dtype=np.float32)
    Wv = np.asarray(Wv, dtype=np.float32)
    Wo = np.asarray(Wo, dtype=np.float32)
    bq = np.asarray(bq, dtype=np.float32)
    bk = np.asarray(bk, dtype=np.float32)
    bv = np.asarray(bv, dtype=np.float32)
    bo = np.asarray(bo, dtype=np.float32)

    xt = np.ascontiguousarray(x.reshape(SF, D_MODEL).T).astype(BF16)
    # 0/1 mask for the diagonal band: allowed iff sk(partition) <= sq(col)
    mask = (
        (np.arange(128)[:, None] <= np.arange(128)[None, :])
        .astype(np.float32)
        .astype(BF16)
    )
    # v-bias passes through attention unchanged (attn rows sum to 1), so it
    # folds into the output bias: bo_eff = bo + Wo @ bv.
    bo_eff = bo + Wo @ bv

    def pack_lhsT(w):  # [1024, 128] k-major -> [128, 8, 128] (p, kk, m)
        return np.ascontiguousarray(
            w.reshape(8, 128, 128).transpose(1, 0, 2)
        ).astype(BF16)

    in_maps = []
    for c in range(N_CORES):
        rows = slice(c * C_LOC, (c + 1) * C_LOC)
        in_maps.append(
            {
                "xt": xt,
                "wq": pack_lhsT((Wq[rows, :] / 8.0).T),
                "wk": pack_lhsT(Wk[rows, :].T),
                "wv": pack_lhsT(Wv[rows, :].T),
                "wo": np.ascontiguousarray(Wo[:, rows].T).astype(BF16),
                "bq": (bq[rows] / 8.0).reshape(128, 1).astype(np.float32),
                "bk": bk[rows].reshape(128, 1).astype(np.float32),
                "mask": mask,
            }
        )
    return in_maps, bo_eff


LAST_RESULTS = None  # BassKernelResults of the most recent run
LAST_RUN_WALL_S = None  # wall seconds of the most recent device dispatch


def kernel(x, Wq, bq, Wk, bk, Wv, bv, Wo, bo):
    global LAST_RESULTS, LAST_RUN_WALL_S
    import time

    from concourse.bass_utils import run_bass_kernel_spmd

    nc = get_program()
    in_maps, bo_eff = shard_inputs(x, Wq, bq, Wk, bk, Wv, bv, Wo, bo)
    trace = bool(os.environ.get("ATTN_KERNEL_TRACE"))
    t0 = time.time()
    res = run_bass_kernel_spmd(
        nc,
        in_maps,
        list(range(N_CORES)),
        trace=trace,
        trace_cores=list(range(N_CORES)) if trace else None,
    )
    LAST_RUN_WALL_S = time.time() - t0
    LAST_RESULTS = res
    acc = np.zeros((SF, D_MODEL), dtype=np.float32)
    for r in res.results:
        acc += np.asarray(r["out"], dtype=np.float32)
    acc += bo_eff[None, :]
    return acc.reshape(B, S, D_MODEL).astype(np.float32)


# revision 8
# speedup vs baseline: 1.4321x; 1.3618x over previous
"""Causal multi-head attention forward on 8 Trainium2 NeuronCores.

Sharding: head-parallel (tensor parallel). 16 heads / 8 cores = 2 heads per
core. Each core:
  - computes q/k/v projections for its 2 heads (weight-sliced),
  - runs causal attention for its 4 (batch, head) pairs,
  - computes the partial output projection over its 128 head-dims.
The host sums the 8 partial outputs (the unshard for a contraction-sharded
matmul) and adds the output bias.

Device design (bf16 matmuls, fp32 PSUM accumulate):
  xT      [1024, 4096]       x transposed (contraction dim on partitions)
  qT, kT  [128, 4096]        head-dims on partitions (q pre-scaled by 1/8)
  v       [128s, 16, 2, 128]  per batch: per sk-tile, per head, an augmented
                              [sk, 128] operand: h0 = [v(64) | ones | zeros],
                              h1 = [ones | zeros | v(64)]. The ones column
                              makes the attention@v matmul emit the softmax
                              denominator as an extra output row (row 64 for
                              h0, row 0 for h1), and places h1's context on
                              partitions 64..127 directly.
  scores  [sk=128, 2, sq<=512] joint PSUM tile (both heads side by side) so
                              one exp instruction covers both heads; the two
                              score matmuls sit on PE row-groups 0-63/64-127
                              and overlap in the array.
Causal handling: fully-masked column ranges are never computed (matmuls and
exp stream columns [r:512] only); the 128-wide diagonal band is handled by
multiplying exp(scores) with a 0/1 mask on the otherwise idle GpSimd engine.
Softmax normalization: reciprocal of the denominator rows on DVE, broadcast
across partitions with a DRAM-bounce DMA, one multiply per head half.
"""

import os

import numpy as np
import ml_dtypes

D_MODEL = 1024
N_HEAD = 16
D_K = 64
B = 2
S = 2048
SF = B * S  # 4096 flattened tokens
N_CORES = 8
HPC = 2  # heads per core
C_LOC = HPC * D_K  # 128 head-dims per core

BF16 = ml_dtypes.bfloat16

_cache = {}


def _build_program():
    """Build + compile the single-core SPMD Bass program (identical on all
    cores; per-core behavior comes entirely from the input tensors)."""
    import concourse.bass as bass  # noqa: F401
    import concourse.tile as tile
    from concourse import bacc, mybir

    fp32 = mybir.dt.float32
    bf16 = mybir.dt.bfloat16
    EXP = mybir.ActivationFunctionType.Exp

    nc = bacc.Bacc(
        "TRN2",
        target_bir_lowering=False,
        debug=False,
        enable_asserts=False,
        num_devices=N_CORES,
    )

    xt_d = nc.dram_tensor("xt", [D_MODEL, SF], bf16, kind="ExternalInput").ap()
    wq_d = nc.dram_tensor("wq", [128, 8, 128], bf16, kind="ExternalInput").ap()
    wk_d = nc.dram_tensor("wk", [128, 8, 128], bf16, kind="ExternalInput").ap()
    wv_d = nc.dram_tensor("wv", [128, 8, 128], bf16, kind="ExternalInput").ap()
    wo_d = nc.dram_tensor("wo", [128, D_MODEL], bf16, kind="ExternalInput").ap()
    bq_d = nc.dram_tensor("bq", [128, 1], fp32, kind="ExternalInput").ap()
    bk_d = nc.dram_tensor("bk", [128, 1], fp32, kind="ExternalInput").ap()
    id_d = nc.dram_tensor("ident", [128, 128], bf16, kind="ExternalInput").ap()
    rm_d = nc.dram_tensor("rmask", [128, 128], bf16, kind="ExternalInput").ap()
    out_d = nc.dram_tensor("out", [SF, D_MODEL], fp32, kind="ExternalOutput").ap()

    with tile.TileContext(nc) as tc:
        with (
            tc.tile_pool(name="persist", bufs=1) as persist,
            tc.tile_pool(name="work", bufs=2) as work,
            tc.tile_pool(name="psum", bufs=1, space="PSUM") as psum,
            tc.tile_pool(name="dscratch", bufs=2, space="DRAM") as dpool,
        ):
            # ---- resident weights/constants -------------------------------
            wq_sb = persist.tile([128, 8, 128], bf16, tag="wq")
            wk_sb = persist.tile([128, 8, 128], bf16, tag="wk")
            wv_sb = persist.tile([128, 8, 128], bf16, tag="wv")
            wo_sb = persist.tile([128, D_MODEL], bf16, tag="wo")
            bq_sb = persist.tile([128, 1], fp32, tag="bq")
            bk_sb = persist.tile([128, 1], fp32, tag="bk")
            id_sb = persist.tile([128, 128], bf16, tag="ident")
            rm_sb = persist.tile([128, 128], bf16, tag="rmask")
            nc.sync.dma_start(out=wq_sb[:], in_=wq_d[:])
            nc.sync.dma_start(out=wk_sb[:], in_=wk_d[:])
            nc.sync.dma_start(out=wv_sb[:], in_=wv_d[:])
            nc.sync.dma_start(out=wo_sb[:], in_=wo_d[:])
            nc.sync.dma_start(out=bq_sb[:], in_=bq_d[:])
            nc.sync.dma_start(out=bk_sb[:], in_=bk_d[:])
            nc.sync.dma_start(out=id_sb[:], in_=id_d[:])
            nc.sync.dma_start(out=rm_sb[:], in_=rm_d[:])

            # ---- x transposed, resident ----------------------------------
            xt_sb = []
            for kk in range(8):
                t = persist.tile([128, SF], bf16, tag=f"xt{kk}", name=f"xt{kk}")
                nc.sync.dma_start(out=t[:], in_=xt_d[kk * 128 : (kk + 1) * 128, :])
                xt_sb.append(t)

            # ---- projection outputs, resident ----------------------------
            qt_sb = persist.tile([128, SF], bf16, tag="qt")
            kt_sb = persist.tile([128, SF], bf16, tag="kt")
            v_sb = []
            for b in range(B):
                t = persist.tile([128, 16, 2, 128], bf16, tag=f"v{b}", name=f"v{b}")
                # h0 block: [v(0:64) | ones col 64 | zeros 65:128]
                nc.vector.memset(t[:, :, 0, 64:65], 1.0)
                nc.vector.memset(t[:, :, 0, 65:128], 0.0)
                # h1 block: [ones col 0 | zeros 1:64 | v(64:128)]
                nc.vector.memset(t[:, :, 1, 0:1], 1.0)
                nc.vector.memset(t[:, :, 1, 1:64], 0.0)
                v_sb.append(t)

            # ---- fused schedule --------------------------------------------
            # Pair t = (projection chunk t) + (o-proj of attention block t-1)
            # + (attention block t). Interleaving keeps the PE stream dense
            # (PE-heavy projection fills the exp-wait gaps of the ACT-heavy
            # attention), which also keeps the HAM clock-gate at 2.4 GHz.
            def proj_chunk(n):
                cs = slice(n * 512, (n + 1) * 512)
                qk = psum.tile([128, 2, 512], fp32, tag="st", bufs=2, name="qk")
                vj = psum.tile([128, 4, 128], fp32, tag="av", bufs=2, name="vj")
                for kk in range(8):
                    st_flag = kk == 0
                    sp_flag = kk == 7
                    nc.tensor.matmul(
                        qk[:, 0, :],
                        lhsT=wq_sb[:, kk, :],
                        rhs=xt_sb[kk][:, cs],
                        start=st_flag,
                        stop=sp_flag,
                    )
                    for ss in (0, 1):
                        t128 = n * 4 + ss
                        # all four v slices live in one PSUM bank: one
                        # accumulation group spans them (start clears the
                        # bank; has_written handles first-touch overwrite)
                        nc.tensor.matmul(
                            vj[:, ss, :],
                            lhsT=xt_sb[kk][:, t128 * 128 : (t128 + 1) * 128],
                            rhs=wv_sb[:, kk, :],
                            start=(kk == 0 and ss == 0),
                            stop=(kk == 7 and ss == 3),
                        )
                    nc.tensor.matmul(
                        qk[:, 1, :],
                        lhsT=wk_sb[:, kk, :],
                        rhs=xt_sb[kk][:, cs],
                        start=st_flag,
                        stop=sp_flag,
                    )
                    for ss in (2, 3):
                        t128 = n * 4 + ss
                        nc.tensor.matmul(
                            vj[:, ss, :],
                            lhsT=xt_sb[kk][:, t128 * 128 : (t128 + 1) * 128],
                            rhs=wv_sb[:, kk, :],
                            start=False,
                            stop=(kk == 7 and ss == 3),
                        )
                nc.vector.tensor_scalar_add(
                    out=qt_sb[:, cs], in0=qk[:, 0, :], scalar1=bq_sb
                )
                nc.vector.tensor_scalar_add(
                    out=kt_sb[:, cs], in0=qk[:, 1, :], scalar1=bk_sb
                )
                for ss in range(4):
                    t128 = n * 4 + ss
                    b, i = divmod(t128, 16)
                    nc.vector.tensor_copy(
                        out=v_sb[b][:, i, 0, 0:64], in_=vj[:, ss, 0:64]
                    )
                    nc.vector.tensor_copy(
                        out=v_sb[b][:, i, 1, 64:128], in_=vj[:, ss, 64:128]
                    )

            def attn_block(b, J):
                """Scores+exp+attention@v for one (batch, 512-query) block.
                Returns the normalized-context tile for the deferred o-proj.
                The attention@v matmuls trail the score matmuls by AV_LAG
                iterations so the PE never waits on the ScalarE exp."""
                AV_LAG = 3
                cb = b * S
                av = psum.tile([128, 2, 512], fp32, tag="av", bufs=2, name="av")
                ntiles = 4 * (J + 1)
                ptiles = {}

                def do_av(i):
                    r = max(0, i * 128 - J * 512)
                    p = ptiles.pop(i)
                    for h in range(2):
                        nc.tensor.matmul(
                            av[:, h, r:512],
                            lhsT=v_sb[b][:, i, h, :],
                            rhs=p[:, h, r:512],
                            start=(i == 0),
                            stop=(i == ntiles - 1),
                        )

                for i in range(ntiles):
                    r = max(0, i * 128 - J * 512)  # 0 except diagonal tiles
                    diag = i >= 4 * J
                    ks = slice(cb + i * 128, cb + (i + 1) * 128)
                    qs = slice(cb + J * 512 + r, cb + (J + 1) * 512)
                    st = psum.tile(
                        [128, 2, 512], fp32, tag="st", bufs=2, name="st"
                    )
                    for h in range(2):
                        hp = slice(h * 64, (h + 1) * 64)
                        nc.tensor.matmul(
                            st[:, h, r:512],
                            lhsT=kt_sb[hp, ks],
                            rhs=qt_sb[hp, qs],
                            start=True,
                            stop=not diag,
                        )
                        if diag:
                            # add -1e30 to the not-allowed part of the
                            # 128-wide diagonal band: st += I.T @ R with R
                            # strictly lower triangular of -1e30 (keeps the
                            # causal mask entirely on the PE)
                            nc.tensor.matmul(
                                st[:, h, r : r + 128],
                                lhsT=id_sb[:],
                                rhs=rm_sb[:],
                                start=False,
                                stop=True,
                            )
                    p = work.tile([128, 2, 512], bf16, tag="p", bufs=6, name="p")
                    nc.scalar.activation(
                        out=p[:, :, r:512], in_=st[:, :, r:512], func=EXP
                    )
                    ptiles[i] = p
                    if i >= AV_LAG:
                        do_av(i - AV_LAG)
                for i in range(max(0, ntiles - AV_LAG), ntiles):
                    do_av(i)

                # Drain the attention output out of PSUM immediately so the
                # av slot frees fast (the normalization chain has DMA latency
                # in it and must not gate PSUM reuse).
                avu = work.tile([128, 2, 512], fp32, tag="avu", bufs=3, name="avu")
                nc.vector.tensor_copy(out=avu[0:65, 0, :], in_=av[0:65, 0, :])
                nc.vector.tensor_copy(out=avu[:, 1, :], in_=av[:, 1, :])

                # softmax denominators: h0 on partition 64, h1 on partition 0.
                # DVE reciprocal costs ~6 cycles/elem/lane, so reshape the
                # [1,512] rows to [128,4] via a DRAM bounce and do one tiny
                # reciprocal. Every consumer of this chain is traced 1-2
                # pairs later so no engine FIFO ever blocks on its latency.
                rdd = dpool.tile([2, 512], fp32, tag="rdd", bufs=3, name="rdd")
                nc.sync.dma_start(out=rdd[0:1, :], in_=avu[64:65, 0, :])
                nc.sync.dma_start(out=rdd[1:2, :], in_=avu[0:1, 1, :])
                dd = work.tile([128, 8], fp32, tag="dd", bufs=3, name="dd")
                nc.sync.dma_start(
                    out=dd.rearrange("p (h m) -> p h m", h=2),
                    in_=rdd[0:2, :].rearrange("h (m p) -> p h m", p=128),
                )
                return avu, dd

            def norm_block(state):
                # one pair after attn_block: reciprocal + broadcast set-up
                avu, dd = state
                ddr = work.tile([128, 8], fp32, tag="ddr", bufs=3, name="ddr")
                nc.vector.reciprocal(out=ddr[:], in_=dd[:])
                rd = dpool.tile([2, 512], fp32, tag="rd", bufs=3, name="rd")
                nc.sync.dma_start(
                    out=rd[0:2, :].rearrange("h (m p) -> p h m", p=128),
                    in_=ddr.rearrange("p (h m) -> p h m", h=2),
                )
                rb = work.tile([128, 512], fp32, tag="rb", bufs=3, name="rb")
                nc.sync.dma_start(
                    out=rb[0:64, :], in_=rd[0:1, :].to_broadcast([64, 512])
                )
                nc.sync.dma_start(
                    out=rb[64:128, :], in_=rd[1:2, :].to_broadcast([64, 512])
                )
                return avu, rb

            def oproj_block(b, J, state):
                # two pairs after attn_block: normalize (on GpSimd — all
                # operands in SBUF) and do the partial output projection.
                avu, rb = state
                ctxt = work.tile([128, 512], bf16, tag="ctx", bufs=2, name="ctxt")
                nc.gpsimd.tensor_mul(
                    out=ctxt[0:64, :], in0=avu[0:64, 0, :], in1=rb[0:64, :]
                )
                nc.gpsimd.tensor_mul(
                    out=ctxt[64:128, :], in0=avu[64:128, 1, :], in1=rb[64:128, :]
                )
                for m in range(4):
                    op = psum.tile([128, 2, 512], fp32, tag="av", bufs=2, name="op")
                    for nn in range(2):
                        nc.tensor.matmul(
                            op[:, nn, :],
                            lhsT=ctxt[:, m * 128 : (m + 1) * 128],
                            rhs=wo_sb[:, nn * 512 : (nn + 1) * 512],
                            start=True,
                            stop=True,
                        )
                    ob = work.tile([128, D_MODEL], fp32, tag="ob", bufs=3, name="ob")
                    nc.vector.tensor_copy(out=ob[:], in_=op[:])
                    row0 = b * S + J * 512 + m * 128
                    nc.sync.dma_start(out=out_d[row0 : row0 + 128, :], in_=ob[:])

            # 3-stage pipeline over the 8 pairs: attn(t) | norm(t-1) |
            # normalize+oproj(t-2), with projection chunk t leading pair t.
            states = {}
            for t in range(10):
                if t < 8:
                    proj_chunk(t)
                if 1 <= t <= 8:
                    states[t - 1] = norm_block(states[t - 1])
                if t >= 2:
                    b, J = divmod(t - 2, 4)
                    oproj_block(b, J, states.pop(t - 2))
                if t < 8:
                    b, J = divmod(t, 4)
                    states[t] = attn_block(b, J)

    nc.compile()
    return nc


def get_program():
    if "nc" not in _cache:
        _cache["nc"] = _build_program()
    return _cache["nc"]


def shard_inputs(x, Wq, bq, Wk, bk, Wv, bv, Wo, bo):
    """Host-side sharding/layout prep. Returns (in_maps, bo_eff)."""
    x = np.asarray(x, dtype=np.float32)
    Wq = np.asarray(Wq, dtype=np.float32)
    Wk = np.asarray(Wk, dtype=np.float32)
    Wv = np.asarray(Wv, dtype=np.float32)
    Wo = np.asarray(Wo, dtype=np.float32)
    bq = np.asarray(bq, dtype=np.float32)
    bk = np.asarray(bk, dtype=np.float32)
    bv = np.asarray(bv, dtype=np.float32)
    bo = np.asarray(bo, dtype=np.float32)

    xt = np.ascontiguousarray(x.reshape(SF, D_MODEL).T).astype(BF16)
    # causal mask for the diagonal band, applied on-device as I.T @ R:
    # R[k, g] = -1e30 where k > g (sk > sq is not allowed)
    ident = np.eye(128, dtype=np.float32).astype(BF16)
    rmask = np.where(
        np.arange(128)[:, None] > np.arange(128)[None, :], -1.0e30, 0.0
    ).astype(np.float32).astype(BF16)
    # v-bias passes through attention unchanged (attn rows sum to 1), so it
    # folds into the output bias: bo_eff = bo + Wo @ bv.
    bo_eff = bo + Wo @ bv

    def pack_lhsT(w):  # [1024, 128] k-major -> [128, 8, 128] (p, kk, m)
        return np.ascontiguousarray(
            w.reshape(8, 128, 128).transpose(1, 0, 2)
        ).astype(BF16)

    in_maps = []
    for c in range(N_CORES):
        rows = slice(c * C_LOC, (c + 1) * C_LOC)
        in_maps.append(
            {
                "xt": xt,
                "wq": pack_lhsT((Wq[rows, :] / 8.0).T),
                "wk": pack_lhsT(Wk[rows, :].T),
                "wv": pack_lhsT(Wv[rows, :].T),
                "wo": np.ascontiguousarray(Wo[:, rows].T).astype(BF16),
                "bq": (bq[rows] / 8.0).reshape(128, 1).astype(np.float32),
                "bk": bk[rows].reshape(128, 1).astype(np.float32),
                "ident": ident,
                "rmask": rmask,
            }
        )
    return in_maps, bo_eff


LAST_RESULTS = None  # BassKernelResults of the most recent run
LAST_RUN_WALL_S = None  # wall seconds of the most recent device dispatch


def kernel(x, Wq, bq, Wk, bk, Wv, bv, Wo, bo):
    global LAST_RESULTS, LAST_RUN_WALL_S
    import time

    from concourse.bass_utils import run_bass_kernel_spmd

    nc = get_program()
    in_maps, bo_eff = shard_inputs(x, Wq, bq, Wk, bk, Wv, bv, Wo, bo)
    trace = bool(os.environ.get("ATTN_KERNEL_TRACE"))
    t0 = time.time()
    res = run_bass_kernel_spmd(
        nc,
        in_maps,
        list(range(N_CORES)),
        trace=trace,
        trace_cores=list(range(N_CORES)) if trace else None,
    )
    LAST_RUN_WALL_S = time.time() - t0
    LAST_RESULTS = res
    acc = np.zeros((SF, D_MODEL), dtype=np.float32)
    for r in res.results:
        acc += np.asarray(r["out"], dtype=np.float32)
    acc += bo_eff[None, :]
    return acc.reshape(B, S, D_MODEL).astype(np.float32)
